# revision 1
# baseline (speedup 1.0000x reference)
"""Trainium2 Bass kernel for nn_DeformCrossAttention2D (sparse_attention).

Sharding: the 2500 query positions (50x50 grid) are split across 8 NeuronCores
as 7 grid rows (350 positions) each; cores whose rows extend past the grid
compute padding the host drops.  Each core redundantly computes the full
q-projection + offset conv + deformable sampling + k/v projections (small),
then runs attention and the CPB relative-position-bias MLP for its i-block.

Attention is computed transposed (j=144 keypoints on partitions, i free).
The CPB MLP (4g x 144j x 350i rows through 2->64->64->2) dominates; layer 1
streams per-(g,j) separable sign-log tables through K=1 matmuls with
broadcast access patterns, layers 2/3 run on the PE with row/col tile
packing, and PSUM evictions (fused relu + bf16 cast) alternate between the
Scalar and Vector engines.  The layer-3 output (2 x rows) is re-laid-out to
(j, i) tiles via a DRAM staging round trip (DMA handles the partition
shuffle; engines cannot cross partitions).
"""
import os
from contextlib import ExitStack

import numpy as np
import ml_dtypes

import concourse.bass as bass
import concourse.tile as tile
import concourse.mybir as mybir
from concourse import bacc
from concourse.bass_utils import run_bass_kernel_spmd

BF = ml_dtypes.bfloat16
F32 = mybir.dt.float32
BF16 = mybir.dt.bfloat16
I16 = mybir.dt.int16
U16 = mybir.dt.uint16
AF = mybir.ActivationFunctionType
ALU = mybir.AluOpType

H = W = 50
DIM = 256
HEADS = 8
DH = 64
G = 4
INNER = 512
C1 = 64
O = 2
NJ = 144
JW = 12
KS = 6
SCALE = DH ** -0.5

NCORE = 8
NIY = 7
IBLK = NIY * W            # 350
NGJ = G * NJ              # 576
NSB_G = 9                 # superblocks per group
SBC = 16                  # chunks per superblock
PADI = NCORE * IBLK       # 2800

DEBUG = os.environ.get("KERNEL_DEBUG", "0") == "1"


def _f(x):
    return np.ascontiguousarray(np.asarray(x), dtype=np.float32)


def _bf(x):
    return np.ascontiguousarray(_f(x).astype(BF))


def host_prep(inputs):
    """Core-independent device inputs (weights + data)."""
    d = {}
    d['wqT'] = _bf(_f(inputs['wq']).T)            # (256,512)
    d['wqTs'] = _bf(_f(inputs['wq']).T * SCALE)
    d['wkT'] = _bf(_f(inputs['wk']).T)
    d['wvT'] = _bf(_f(inputs['wv']).T)
    d['woutT'] = _bf(_f(inputs['wout']).T)        # (512,256)

    w1 = _f(inputs['w_off1'])                     # (128,1,6,6)
    dconv = np.zeros((36 * 128, 128), np.float32)
    for dy in range(KS):
        for dx in range(KS):
            t = dy * KS + dx
            np.fill_diagonal(dconv[t * 128:(t + 1) * 128], w1[:, 0, dy, dx])
    d['dconv'] = _bf(dconv)
    d['boff1'] = _f(inputs['b_off1']).reshape(128, 1)
    d['woff2T'] = _bf(_f(inputs['w_off2'])[:, :, 0, 0].T)      # (128,2)

    cw1 = _f(inputs['cpb_w1'])
    w1x4 = np.zeros((128, C1), np.float32)
    w1y4 = np.zeros((128, C1), np.float32)
    for rb in range(4):
        w1x4[32 * rb] = cw1[0]
        w1y4[32 * rb] = cw1[1]
    d['w1x4'] = _bf(w1x4)
    d['w1y4'] = _bf(w1y4)
    d['w2dup'] = _bf(np.concatenate([_f(inputs['cpb_w2'])] * 2, 0))
    w3 = _f(inputs['cpb_w3'])
    w3duo = np.zeros((128, 4), np.float32)
    w3duo[0:64, 0:2] = w3
    w3duo[64:128, 2:4] = w3
    d['w3duo'] = _bf(w3duo)
    d['b1dup'] = _f(np.concatenate([_f(inputs['cpb_b1'])] * 2)).reshape(128, 1)
    d['b2dup'] = _f(np.concatenate([_f(inputs['cpb_b2'])] * 2)).reshape(128, 1)
    d['b3rep'] = _f(np.broadcast_to(_f(inputs['cpb_b3']), (128, 2)))
    bout = _f(inputs['bout'])
    d['boutrep'] = _f(np.stack([bout[0:128], bout[128:256]], 1))

    x1 = _f(inputs['x1'])[0]
    x2 = _f(inputs['x2'])[0]
    d['f1bf'] = _bf(x1[:, 1:])                    # (256,2500)
    d['f2f'] = _f(x2[:, 1:])                      # (256,2500) fp32 for gather
    return d


def host_core(inputs, core):
    x1 = _f(inputs['x1'])[0]
    f1p = np.zeros((DIM, PADI), np.float32)
    f1p[:, :2500] = x1[:, 1:]
    iy0 = core * NIY
    gyn = 2.0 * (iy0 + np.arange(NIY, dtype=np.float32)) / (H - 1) - 1.0
    return {
        'f1blk': _bf(f1p[:, core * IBLK:(core + 1) * IBLK]),
        'gyc': _f(np.broadcast_to(gyn, (128, NIY))),
    }


_NPDT = {np.dtype(np.float32): F32, np.dtype(BF): BF16,
         np.dtype(np.uint16): U16, np.dtype(np.int16): I16}


def _consts():
    """Static (input-independent) arrays, embedded in the NEFF."""
    c = {}
    gxn = 2.0 * np.arange(W, dtype=np.float32) / (W - 1) - 1.0
    c['gxc'] = _f(np.broadcast_to(gxn, (128, W)))
    g12x, g12y = np.meshgrid(np.arange(JW, dtype=np.float32),
                             np.arange(JW, dtype=np.float32), indexing='xy')
    base = np.stack([g12x.reshape(-1), g12y.reshape(-1)], 0)
    c['gridjc'] = _f((2.0 / (JW - 1)) * np.tile(base, (1, G)) - 1.0)
    c['onesrow'] = _bf(np.ones((128, 128)))
    c['onescol'] = _bf(np.ones((128, 1)))
    c['idn'] = _f(np.eye(128, dtype=np.float32))
    perm = np.zeros((128, 4 * 36), np.uint16)
    for g in range(G):
        for l in range(576):
            t, j = divmod(l, 144)
            p, fi = l % 16, l // 16
            for blk in range(8):
                perm[16 * blk + p, g * 36 + fi] = g * 576 + t * 144 + j
    c['idxperm'] = perm
    return c


def build_nc(shapes):
    nc = bacc.Bacc("TRN2", target_bir_lowering=False, debug=False)
    di = {k: nc.dram_tensor(k, list(v.shape), _NPDT[v.dtype],
                            kind="ExternalInput")
          for k, v in shapes.items()}
    out_d = nc.dram_tensor("out", [DIM, IBLK], F32, kind="ExternalOutput")
    cdata = _consts()
    ci = {k: nc.inline_tensor(v, name="c_" + k) for k, v in cdata.items()}
    dbt = nc.dram_tensor("dbt", [HEADS, NJ, IBLK], BF16, kind="Internal")
    dbg = {}

    def dbgt(name, shape, dt=F32):
        dbg[name] = nc.dram_tensor("dbg_" + name, list(shape), dt,
                                   kind="ExternalOutput")
        return dbg[name]

    with ExitStack() as ctx:
        tc = ctx.enter_context(tile.TileContext(nc))
        cst = ctx.enter_context(tc.tile_pool(name="cst", bufs=1))
        per = ctx.enter_context(tc.tile_pool(name="per", bufs=1))

        def load(name, pool=cst, rows=None, src=None):
            a = shapes[name] if src is None else cdata[name]
            h = (di if src is None else ci)[name]
            r0, r1 = (0, a.shape[0]) if rows is None else rows
            t = pool.tile([r1 - r0] + list(a.shape[1:]), _NPDT[a.dtype],
                          name=f"ld_{name}_{r0}", tag=f"ld_{name}_{r0}")
            nc.sync.dma_start(t[:], h[r0:r1])
            return t

        def loadc(name, pool=cst, rows=None):
            return load(name, pool, rows, src=True)

        # ---- weights / constants (live whole kernel) ----
        WQT = [load('wqT', cst, (128 * i, 128 * (i + 1))) for i in range(2)]
        WQS = [load('wqTs', cst, (128 * i, 128 * (i + 1))) for i in range(2)]
        WKT = [load('wkT', cst, (128 * i, 128 * (i + 1))) for i in range(2)]
        WVT = [load('wvT', cst, (128 * i, 128 * (i + 1))) for i in range(2)]
        WOT = [load('woutT', cst, (128 * i, 128 * (i + 1))) for i in range(4)]
        DCV = [load('dconv', cst, (128 * t, 128 * (t + 1))) for t in range(36)]
        WOFF2 = load('woff2T')
        W1X4 = load('w1x4')
        W1Y4 = load('w1y4')
        W2D = load('w2dup')
        W3D = load('w3duo')
        B1 = load('b1dup')
        B2 = load('b2dup')
        BOFF1 = load('boff1')
        B3R = load('b3rep')
        BOUTR = load('boutrep')
        GYC = load('gyc')
        GXC = loadc('gxc')
        GRIDJ = loadc('gridjc')
        ONESR = loadc('onesrow')
        ONESC = loadc('onescol')
        IDN = loadc('idn')

        # ---- tiles that span phases ----
        SXT = per.tile([128, NGJ * W], BF16)
        SYT = per.tile([128, NGJ * NIY], BF16)
        QB = [per.tile([128, IBLK], BF16, tag=f"qblk{i}", name=f"qblk{i}")
              for i in range(4)]
        KH = [per.tile([128, NJ], BF16, tag=f"kh{i}", name=f"kh{i}")
              for i in range(4)]
        VT0 = per.tile([128, INNER], BF16)
        VT1 = per.tile([16, INNER], BF16)
        OFFPRE = per.tile([2, NGJ], F32)

        # =========== phase A: q projection + offset conv ===========
        with tc.tile_pool(name="pA", bufs=1) as pA, \
             tc.tile_pool(name="psA", bufs=2, space="PSUM") as psA:
            F1 = [load('f1bf', pA, (128 * i, 128 * (i + 1)))
                  for i in range(2)]
            F1K = [load('f1blk', pA, (128 * i, 128 * (i + 1)))
                   for i in range(2)]
            QP = []
            GEL = []
            for mt in range(4):
                qp = pA.tile([128, 52 * 52], BF16, tag=f"qpad{mt}",
                             name=f"qpad{mt}")
                nc.vector.memset(qp[:], 0.0)
                QP.append(qp)
                for nq in range(5):
                    pq = psA.tile([128, 512], F32, tag="psq")
                    for kt in range(2):
                        nc.tensor.matmul(
                            pq[:, 0:500], WQT[kt][:, 128 * mt:128 * (mt + 1)],
                            F1[kt][:, 500 * nq:500 * (nq + 1)],
                            start=(kt == 0), stop=(kt == 1))
                    dst = qp[:].rearrange("p (r c) -> p r c", c=52)[
                        :, 1 + 10 * nq:11 + 10 * nq, 1:51]
                    if nq % 2 == 0:
                        nc.scalar.activation(dst, pq[:, 0:500], AF.Copy)
                    else:
                        nc.vector.tensor_copy(dst, pq[:, 0:500])
                pqb = psA.tile([128, 512], F32, tag="psqb")
                for kt in range(2):
                    nc.tensor.matmul(
                        pqb[:, 0:IBLK], WQS[kt][:, 128 * mt:128 * (mt + 1)],
                        F1K[kt][:], start=(kt == 0), stop=(kt == 1))
                nc.scalar.activation(QB[mt][:], pqb[:, 0:IBLK], AF.Copy)
            if DEBUG:
                nc.sync.dma_start(dbgt('qpad0', (128, 2704), BF16)[:],
                                  QP[0][:])

            for g in range(G):
                pc = psA.tile([128, 512], F32, tag="psconv")
                for t in range(36):
                    dy, dx = divmod(t, KS)
                    src = QP[g][:].rearrange(
                        "p (rb rf cb cf) -> p rb rf cb cf",
                        rf=4, cb=13, cf=4)[
                        :, dy // 4:dy // 4 + 12, dy % 4,
                        dx // 4:dx // 4 + 12, dx % 4]
                    nc.tensor.matmul(pc[:, 0:NJ], DCV[t][:], src,
                                     start=(t == 0), stop=(t == 35))
                gel = pA.tile([128, NJ], BF16, tag=f"gel{g}", name=f"gel{g}")
                nc.scalar.activation(gel[:], pc[:, 0:NJ], AF.Gelu, bias=BOFF1[:])
                GEL.append(gel)

            for bank in range(2):
                po = psA.tile([2, 512], F32, tag="psoff")
                for gg in range(2):
                    g = 2 * bank + gg
                    nc.tensor.matmul(po[:, 144 * gg:144 * (gg + 1)],
                                     WOFF2[:], GEL[g][:],
                                     start=(gg == 0), stop=(gg == 1))
                nc.scalar.activation(OFFPRE[:, 288 * bank:288 * (bank + 1)],
                                     po[:, 0:288], AF.Copy)
        if DEBUG:
            nc.sync.dma_start(dbgt('offpre', (2, NGJ))[:], OFFPRE[:])

        # ====== phases B/C: offsets, sampling, tables, gather, k/v ======
        KV = None
        with tc.tile_pool(name="pB", bufs=1) as pB, \
             tc.tile_pool(name="pBs", bufs=1) as pBs:
            VS = pB.tile([2, NGJ], F32)
            TH = pBs.tile([2, NGJ], F32, tag="th")
            nc.scalar.activation(TH[:], OFFPRE[:], AF.Tanh)
            nc.vector.scalar_tensor_tensor(VS[:], TH[:], 8.0 / (JW - 1),
                                           GRIDJ[:], op0=ALU.mult,
                                           op1=ALU.add)
            if DEBUG:
                nc.sync.dma_start(dbgt('vs', (2, NGJ))[:], VS[:])

            XY = pBs.tile([2, NGJ], F32, tag="xy")
            nc.vector.tensor_scalar(XY[:], VS[:], 25.0, 24.5, op0=ALU.mult,
                                    op1=ALU.add)
            XS = pBs.tile([2, NGJ], F32, tag="xs")
            nc.vector.tensor_scalar(XS[:], XY[:], 64.0, None, op0=ALU.add)
            YI = pBs.tile([2, NGJ], mybir.dt.int32, tag="yi")
            nc.vector.tensor_copy(YI[:], XS[:])
            YF = pBs.tile([2, NGJ], F32, tag="yf")
            nc.vector.tensor_copy(YF[:], YI[:])
            GT = pBs.tile([2, NGJ], F32, tag="gt")
            nc.vector.tensor_tensor(GT[:], YF[:], XS[:], op=ALU.is_gt)
            FL = pBs.tile([2, NGJ], F32, tag="fl")
            nc.vector.tensor_tensor(FL[:], YF[:], GT[:], op=ALU.subtract)
            X0 = pB.tile([2, NGJ], F32)
            nc.vector.tensor_scalar(X0[:], FL[:], -64.0, None, op0=ALU.add)
            FR = pB.tile([2, NGJ], F32)
            nc.vector.tensor_tensor(FR[:], XY[:], X0[:], op=ALU.subtract)
            CL0 = pB.tile([2, NGJ], F32)
            nc.vector.tensor_scalar(CL0[:], X0[:], 0.0, 49.0, op0=ALU.max,
                                    op1=ALU.min)
            CL1 = pB.tile([2, NGJ], F32)
            nc.vector.tensor_scalar(CL1[:], X0[:], 1.0, 0.0, op0=ALU.add,
                                    op1=ALU.max)
            nc.vector.tensor_scalar(CL1[:], CL1[:], 49.0, None, op0=ALU.min)
            V0 = pB.tile([2, NGJ], F32)
            TMP = pBs.tile([2, NGJ], F32, tag="tmp")
            nc.vector.tensor_scalar(V0[:], X0[:], -0.5, None, op0=ALU.is_gt)
            nc.vector.tensor_scalar(TMP[:], X0[:], 49.5, None, op0=ALU.is_lt)
            nc.vector.tensor_tensor(V0[:], V0[:], TMP[:], op=ALU.mult)
            V1 = pB.tile([2, NGJ], F32)
            nc.vector.tensor_scalar(V1[:], X0[:], -1.5, None, op0=ALU.is_gt)
            nc.vector.tensor_scalar(TMP[:], X0[:], 48.5, None, op0=ALU.is_lt)
            nc.vector.tensor_tensor(V1[:], V1[:], TMP[:], op=ALU.mult)
            U0 = pB.tile([2, NGJ], F32)
            nc.vector.tensor_scalar(U0[:], FR[:], -1.0, 1.0, op0=ALU.mult,
                                    op1=ALU.add)
            nc.vector.tensor_tensor(U0[:], U0[:], V0[:], op=ALU.mult)
            U1 = pB.tile([2, NGJ], F32)
            nc.vector.tensor_tensor(U1[:], FR[:], V1[:], op=ALU.mult)

            CF = pB.tile([1, 4 * NGJ], F32)
            UFl = pB.tile([1, 4 * NGJ], F32)
            for r in range(2):
                nc.sync.dma_start(CF[0:1, r * NGJ:(r + 1) * NGJ],
                                  CL0[r:r + 1, :])
                nc.sync.dma_start(CF[0:1, (2 + r) * NGJ:(3 + r) * NGJ],
                                  CL1[r:r + 1, :])
                nc.sync.dma_start(UFl[0:1, r * NGJ:(r + 1) * NGJ],
                                  U0[r:r + 1, :])
                nc.sync.dma_start(UFl[0:1, (2 + r) * NGJ:(3 + r) * NGJ],
                                  U1[r:r + 1, :])

            IDXF = pB.tile([1, 4 * NGJ], F32)
            WTF = pB.tile([1, 4 * NGJ], F32)
            for ty in range(2):
                for tx in range(2):
                    t = 2 * ty + tx
                    idst = IDXF[0:1, :].rearrange(
                        "o (g j t) -> o t g j", g=G, t=4)[:, t]
                    wdst = WTF[0:1, :].rearrange(
                        "o (g j t) -> o t g j", g=G, t=4)[:, t]
                    cy = CF[0:1, (2 * ty + 1) * NGJ:(2 * ty + 2) * NGJ]
                    cx = CF[0:1, (2 * tx) * NGJ:(2 * tx + 1) * NGJ]
                    nc.vector.scalar_tensor_tensor(
                        idst, cy.rearrange("o (g j) -> o g j", g=G), 50.0,
                        cx.rearrange("o (g j) -> o g j", g=G),
                        op0=ALU.mult, op1=ALU.add)
                    uy = UFl[0:1, (2 * ty + 1) * NGJ:(2 * ty + 2) * NGJ]
                    ux = UFl[0:1, (2 * tx) * NGJ:(2 * tx + 1) * NGJ]
                    nc.vector.tensor_tensor(
                        wdst, uy.rearrange("o (g j) -> o g j", g=G),
                        ux.rearrange("o (g j) -> o g j", g=G), op=ALU.mult)
            IDXSN = pB.tile([1, 4 * NGJ], F32)
            nc.vector.tensor_scalar(IDXSN[:], IDXF[:], 0.4, None, op0=ALU.add)
            if DEBUG:
                nc.sync.dma_start(dbgt('idxf', (1, 4 * NGJ))[:], IDXF[:])
                nc.sync.dma_start(dbgt('wtf', (1, 4 * NGJ))[:], WTF[:])

            # wrapped idx layout per group via PE transpose of (36,16) stage
            IDXW = pB.tile([64, 4 * 36], I16)
            IDXS36 = pBs.tile([36, 16], F32, tag="idxs36")
            with tc.tile_pool(name="psW", bufs=2, space="PSUM") as psW:
                for g in range(G):
                    stg36 = pBs.tile([36, 16], F32, tag="idxs36")
                    nc.sync.dma_start(
                        stg36[:],
                        IDXSN[0:1, 576 * g:576 * (g + 1)].rearrange(
                            "o (s p) -> o s p", p=16))
                    ptw = psW.tile([16, 512], F32, tag="ptw")
                    nc.tensor.transpose(ptw[:, 0:36], stg36[:],
                                        IDN[0:36, 0:36])
                    nc.scalar.activation(IDXW[0:16, 36 * g:36 * (g + 1)],
                                         ptw[:, 0:36], AF.Copy)
            for rep in range(1, 4):
                nc.sync.dma_start(IDXW[16 * rep:16 * (rep + 1), :],
                                  IDXW[0:16, :])

            VSP = pB.tile([128, 10], F32)
            nc.vector.memset(VSP[:], 0.0)
            with tc.tile_pool(name="psT", bufs=2, space="PSUM") as psT:
                for t in range(5):
                    wt_ = min(128, NGJ - 128 * t)
                    pt = psT.tile([128, 512], F32, tag="pst")
                    nc.tensor.transpose(pt[0:wt_, 0:2],
                                        VS[:, 128 * t:128 * t + wt_],
                                        IDN[0:2, 0:2])
                    dst = VSP[0:wt_, :].rearrange("p (a b) -> p a b",
                                                  b=5)[:, :, t]
                    nc.scalar.activation(dst, pt[0:wt_, 0:2], AF.Copy)

            SXW = pB.tile([128, 5 * W], F32)
            SYW = pB.tile([128, 5 * NIY], F32)
            for t in range(5):
                for (dstw, srcc, n, col) in ((SXW, GXC, W, t),
                                             (SYW, GYC, NIY, 5 + t)):
                    dst = dstw[:, t * n:(t + 1) * n]
                    nc.vector.tensor_scalar(dst, srcc[:, 0:n],
                                            VSP[:, col:col + 1], None,
                                            op0=ALU.subtract)
                    sgn = pBs.tile([128, W], F32, tag="sgn")
                    nc.scalar.activation(sgn[:, 0:n], dst, AF.Sign)
                    nc.scalar.activation(dst, dst, AF.Abs)
                    nc.scalar.activation(dst, dst, AF.Ln, bias=1.0)
                    nc.vector.tensor_tensor(dst, dst, sgn[:, 0:n],
                                            op=ALU.mult)
            SXB = pB.tile([128, 5 * W], BF16)
            nc.vector.tensor_copy(SXB[:], SXW[:])
            SYB = pB.tile([128, 5 * NIY], BF16)
            nc.vector.tensor_copy(SYB[:], SYW[:])

            for t in range(4):
                nc.sync.dma_start(
                    SXT[0:1, t * 128 * W:(t + 1) * 128 * W].rearrange(
                        "o (p x) -> o p x", p=128),
                    SXB[:, t * W:(t + 1) * W])
                nc.sync.dma_start(
                    SYT[0:1, t * 128 * NIY:(t + 1) * 128 * NIY].rearrange(
                        "o (p x) -> o p x", p=128),
                    SYB[:, t * NIY:(t + 1) * NIY])
            nc.sync.dma_start(
                SXT[0:1, 4 * 128 * W:].rearrange("o (p x) -> o p x", p=64),
                SXB[0:64, 4 * W:5 * W])
            nc.sync.dma_start(
                SYT[0:1, 4 * 128 * NIY:].rearrange("o (p x) -> o p x", p=64),
                SYB[0:64, 4 * NIY:5 * NIY])
            for rb in range(1, 4):
                nc.sync.dma_start(SXT[32 * rb:32 * rb + 1, :], SXT[0:1, :])
                nc.sync.dma_start(SYT[32 * rb:32 * rb + 1, :], SYT[0:1, :])
            if DEBUG:
                nc.sync.dma_start(dbgt('sxt', (1, NGJ * W), BF16)[:],
                                  SXT[0:1, :])
                nc.sync.dma_start(dbgt('syt', (1, NGJ * NIY), BF16)[:],
                                  SYT[0:1, :])

            # gather + kv + projections
            KV = [pB.tile([128, NJ], BF16, tag=f"kv{i}", name=f"kv{i}")
                  for i in range(2)]
            for g in range(G):
                F2gt = pBs.tile([64, 2500], F32, tag="f2g", name="f2g")
                nc.sync.dma_start(F2gt[:], di['f2f'][64 * g:64 * (g + 1)])
                G4 = pBs.tile([64, 4 * NJ], F32, tag="g4")
                nc.gpsimd.ap_gather(G4[:], F2gt[:].unsqueeze(2),
                                    IDXW[0:64, 36 * g:36 * (g + 1)],
                                    channels=64, num_elems=2500, d=1,
                                    num_idxs=4 * NJ)
                WTB = pBs.tile([64, 4 * NJ], F32, tag="wtb")
                nc.gpsimd.partition_broadcast(
                    WTB[:], WTF[0:1, 576 * g:576 * (g + 1)])
                E1 = pBs.tile([64, 4 * NJ], F32, tag="e1")
                nc.vector.tensor_tensor(E1[:], G4[:], WTB[:], op=ALU.mult)
                E1v = E1[:].rearrange("p (j t) -> p j t", t=4)
                T0 = pBs.tile([64, NJ], F32, tag="t0")
                nc.vector.tensor_tensor(T0[:], E1v[:, :, 0], E1v[:, :, 1],
                                        op=ALU.add)
                T1 = pBs.tile([64, NJ], F32, tag="t1")
                nc.vector.tensor_tensor(T1[:], E1v[:, :, 2], E1v[:, :, 3],
                                        op=ALU.add)
                dst = KV[g // 2][64 * (g % 2):64 * (g % 2) + 64, :]
                nc.vector.tensor_tensor(dst, T0[:], T1[:], op=ALU.add)
            if DEBUG:
                nc.sync.dma_start(dbgt('kv0', (128, NJ), BF16)[:], KV[0][:])
                nc.sync.dma_start(dbgt('kv1', (128, NJ), BF16)[:], KV[1][:])

            with tc.tile_pool(name="psC", bufs=3, space="PSUM") as psC:
                for mt in range(4):
                    pk = psC.tile([128, 512], F32, tag="psk")
                    for kt in range(2):
                        nc.tensor.matmul(pk[:, 0:NJ],
                                         WKT[kt][:, 128 * mt:128 * (mt + 1)],
                                         KV[kt][:], start=(kt == 0),
                                         stop=(kt == 1))
                    nc.scalar.activation(KH[mt][:], pk[:, 0:NJ], AF.Copy)
                for jm, vt, jn in ((0, VT0, 128), (1, VT1, 16)):
                    pv = psC.tile([jn, INNER], F32, tag="psv")
                    for kt in range(2):
                        nc.tensor.matmul(
                            pv[:], KV[kt][:, 128 * jm:128 * jm + jn],
                            WVT[kt][:], start=(kt == 0), stop=(kt == 1))
                    nc.scalar.activation(vt[:], pv[:], AF.Copy)
            if DEBUG:
                nc.sync.dma_start(dbgt('kh0', (128, NJ), BF16)[:], KH[0][:])
                nc.sync.dma_start(dbgt('vt0', (128, INNER), BF16)[:],
                                  VT0[:])

        # =========== phases D-G ===========
        KTAIL = os.environ.get('KERNEL_TAIL', '1') == '1'
        if not KTAIL:
            with tc.tile_pool(name="pZ", bufs=1) as pZ:
                zt = pZ.tile([128, IBLK], F32)
                nc.vector.memset(zt[:], 0.0)
                for mt in range(2):
                    nc.sync.dma_start(out_d[128 * mt:128 * (mt + 1), :],
                                      zt[:])
        if KTAIL:
         with tc.tile_pool(name="pD", bufs=1) as pD, \
              tc.tile_pool(name="pDs", bufs=2) as pDs:
             SIM0 = [pD.tile([128, IBLK], F32, tag=f"sim0_{h}",
                             name=f"sim0_{h}") for h in range(8)]
             SIM1 = [pD.tile([16, IBLK], F32, tag=f"sim1_{h}",
                             name=f"sim1_{h}") for h in range(8)]
             with tc.tile_pool(name="psS", bufs=4, space="PSUM") as psS:
                 for h in range(8):
                     kt, half = h // 2, 64 * (h % 2)
                     p0 = psS.tile([128, 512], F32, tag="pss0")
                     nc.tensor.matmul(p0[:, 0:IBLK], KH[kt][half:half + 64, 0:128],
                                      QB[kt][half:half + 64, :],
                                      start=True, stop=True,
                                      tile_position=(half, 0))
                     p1 = psS.tile([16, 512], F32, tag="pss1")
                     nc.tensor.matmul(p1[:, 0:IBLK], KH[kt][half:half + 64, 128:NJ],
                                      QB[kt][half:half + 64, :],
                                      start=True, stop=True,
                                      tile_position=(half, 0))
                     if h % 2 == 0:
                         nc.scalar.activation(SIM0[h][:], p0[:, 0:IBLK], AF.Copy)
                         nc.scalar.activation(SIM1[h][:], p1[:, 0:IBLK], AF.Copy)
                     else:
                         nc.vector.tensor_copy(SIM0[h][:], p0[:, 0:IBLK])
                         nc.vector.tensor_copy(SIM1[h][:], p1[:, 0:IBLK])
             if DEBUG:
                 nc.sync.dma_start(dbgt('sim0h0', (128, IBLK))[:],
                                   SIM0[0][:])

             # CPB MLP
             with tc.tile_pool(name="psL1", bufs=1, space="PSUM") as psL1p, \
                  tc.tile_pool(name="psL2", bufs=1, space="PSUM") as psL2p, \
                  tc.tile_pool(name="psL3", bufs=1, space="PSUM") as psL3p, \
                  tc.tile_pool(name="h1p", bufs=2) as h1p, \
                  tc.tile_pool(name="h2p", bufs=4) as h2p, \
                  tc.tile_pool(name="stgp", bufs=2) as stgp:
                 for sb in range(int(os.environ.get('KERNEL_NSB', G * NSB_G))):
                     g, tloc = divmod(sb, NSB_G)
                     psL3X = psL3p.tile([128, 512], F32, tag="l3x")
                     psL3Y = psL3p.tile([128, 512], F32, tag="l3y")
                     H1S = [None, None]
                     for sw in range(2):
                         ps1 = psL1p.tile([128, 2048], F32, tag="l1")
                         for b in range(4):
                             for cp in range(2):
                                 gj = SBC * sb + 8 * sw + 2 * b + cp
                                 outap = ps1[64 * cp:64 * cp + 64,
                                             512 * b:512 * b + IBLK]
                                 rx = SXT[32 * b:32 * b + 1,
                                          W * gj:W * (gj + 1)].unsqueeze(1) \
                                     .broadcast_to([1, NIY, W])
                                 nc.tensor.matmul(
                                     outap, W1X4[32 * b:32 * b + 1, :], rx,
                                     start=True, stop=False,
                                     tile_position=(32 * b, 64 * cp))
                                 ry = SYT[32 * b:32 * b + 1,
                                          NIY * gj:NIY * (gj + 1)] \
                                     .unsqueeze(2).broadcast_to([1, NIY, W])
                                 nc.tensor.matmul(
                                     outap, W1Y4[32 * b:32 * b + 1, :], ry,
                                     start=False, stop=True,
                                     tile_position=(32 * b, 64 * cp))
                         h1s = h1p.tile([128, 4 * IBLK], BF16, tag="h1s")
                         H1S[sw] = h1s
                         src = ps1[:].rearrange("p (b x) -> p b x", b=4)[
                             :, :, 0:IBLK]
                         nc.scalar.activation(
                             h1s[:, 0:2 * IBLK].rearrange(
                                 "p (b x) -> p b x", b=2),
                             src[:, 0:2, :], AF.Relu, bias=B1[:])
                         nc.vector.tensor_scalar(
                             h1s[:, 2 * IBLK:].rearrange(
                                 "p (b x) -> p b x", b=2),
                             src[:, 2:4, :], B1[:], 0.0, op0=ALU.add,
                             op1=ALU.max)
                     for qq in range(4):
                         sw, qh = divmod(qq, 2)
                         h1s = H1S[sw]
                         ps2x = psL2p.tile([128, 512], F32, tag="l2x")
                         ps2y = psL2p.tile([128, 512], F32, tag="l2y")
                         for w_ in range(4):
                             cw = 4 * qh + w_
                             rbase = 64 * (cw % 2)
                             rhs = h1s[rbase:rbase + 64,
                                       IBLK * (cw // 2):IBLK * (cw // 2)
                                       + IBLK]
                             pdst = ps2x if w_ % 2 == 0 else ps2y
                             cbase = 64 * (w_ // 2)
                             nc.tensor.matmul(
                                 pdst[cbase:cbase + 64, 0:IBLK],
                                 W2D[rbase:rbase + 64, :], rhs,
                                 start=True, stop=True,
                                 tile_position=(rbase, cbase))
                         h2x = h2p.tile([128, IBLK], BF16, tag="h2x")
                         h2y = h2p.tile([128, IBLK], BF16, tag="h2y")
                         nc.scalar.activation(h2x[:], ps2x[:, 0:IBLK],
                                              AF.Relu, bias=B2[:])
                         nc.vector.tensor_scalar(h2y[:], ps2y[:, 0:IBLK],
                                                 B2[:], 0.0, op0=ALU.add,
                                                 op1=ALU.max)
                         nc.tensor.matmul(psL3X[32 * qq:32 * qq + 4, 0:IBLK],
                                          W3D[:], h2x[:], start=True,
                                          stop=True,
                                          tile_position=(0, 32 * qq))
                         nc.tensor.matmul(psL3Y[32 * qq:32 * qq + 4, 0:IBLK],
                                          W3D[:], h2y[:], start=True,
                                          stop=True,
                                          tile_position=(0, 32 * qq))
                     stg = stgp.tile([128, 2 * IBLK], BF16, tag="stg")
                     nc.scalar.activation(stg[:, 0:IBLK], psL3X[:, 0:IBLK],
                                          AF.Copy)
                     nc.vector.tensor_copy(stg[:, IBLK:], psL3Y[:, 0:IBLK])
                     for qq in range(4):
                         for hh in range(2):
                             p0_ = 32 * qq + 2 * hh
                             j0 = 16 * tloc + 4 * qq + 2 * hh
                             nc.sync.dma_start(
                                 dbt[2 * g:2 * g + 2, j0:j0 + 2, :],
                                 stg[p0_:p0_ + 2, :].rearrange(
                                     "p (y x) -> p y x", y=2))

             # attention tail
             BT0 = [pD.tile([128, IBLK], BF16, tag=f"bt0_{i}",
                            name=f"bt0_{i}") for i in range(8)]
             BT1 = [pD.tile([16, IBLK], BF16, tag=f"bt1_{i}",
                            name=f"bt1_{i}") for i in range(8)]
             for hd in range(HEADS):
                 nc.sync.dma_start(BT0[hd][:], dbt[hd:hd + 1, 0:128, :])
                 nc.sync.dma_start(BT1[hd][:], dbt[hd:hd + 1, 128:NJ, :])
             if DEBUG:
                 nc.sync.dma_start(dbgt('bt0h0', (128, IBLK), BF16)[:],
                                   BT0[0][:])

             E0 = [pD.tile([128, IBLK], BF16, tag=f"e0_{h}", name=f"e0_{h}")
                   for h in range(8)]
             E1T = [pD.tile([16, IBLK], BF16, tag=f"e1_{h}", name=f"e1t_{h}")
                    for h in range(8)]
             for h in range(HEADS):
                 g, o = divmod(h, 2)
                 sad = pDs.tile([128, IBLK], F32, tag="sad")
                 nc.vector.tensor_tensor(sad[:], SIM0[h][:], BT0[h][:],
                                         op=ALU.add)
                 nc.scalar.activation(E0[h][:], sad[:], AF.Exp,
                                      bias=B3R[0:128, o:o + 1])
                 sad1 = pDs.tile([16, IBLK], F32, tag="sad1")
                 nc.vector.tensor_tensor(sad1[:], SIM1[h][:], BT1[h][:],
                                         op=ALU.add)
                 nc.scalar.activation(E1T[h][:], sad1[:], AF.Exp,
                                      bias=B3R[0:16, o:o + 1])
             if DEBUG:
                 nc.sync.dma_start(dbgt('e0h0', (128, IBLK), BF16)[:],
                                   E0[0][:])

             RCP = [pD.tile([128, IBLK], F32, tag=f"rcp{i}", name=f"rcp{i}")
                    for i in range(2)]
             RCB = [pD.tile([128, IBLK], BF16, tag=f"rcb{i}", name=f"rcb{i}")
                    for i in range(2)]
             ANRM = [pD.tile([128, IBLK], BF16, tag=f"an{i}", name=f"an{i}")
                     for i in range(4)]
             with tc.tile_pool(name="psF", bufs=2, space="PSUM") as psF:
                 for bk in range(2):
                     pd = psF.tile([128, 512], F32, tag="psden")
                     nc.vector.memset(pd[:], 1.0)
                     for hh in range(4):
                         h = 4 * bk + hh
                         nc.tensor.matmul(pd[32 * hh:32 * hh + 1, 0:IBLK],
                                          ONESC[0:128, :], E0[h][:],
                                          start=True, stop=False,
                                          tile_position=(0, 32 * hh))
                         nc.tensor.matmul(pd[32 * hh:32 * hh + 1, 0:IBLK],
                                          ONESC[0:16, :], E1T[h][:],
                                          start=False, stop=True,
                                          tile_position=(0, 32 * hh))
                     nc.scalar.activation(RCP[bk][:], pd[:, 0:IBLK], AF.Copy)
                     with nc.allow_low_precision(
                             reason="attn denominators in bf16"):
                         nc.vector.reciprocal(RCB[bk][:], RCP[bk][:])
                 for hp in range(4):
                     pa = psF.tile([128, 512], F32, tag="psa")
                     prc = psF.tile([128, 512], F32, tag="psrc")
                     for hh in range(2):
                         h = 2 * hp + hh
                         nc.tensor.matmul(pa[64 * hh:64 * hh + 64, 0:IBLK],
                                          VT0[:, 64 * h:64 * h + 64],
                                          E0[h][:], start=True, stop=False,
                                          tile_position=(0, 64 * hh))
                         nc.tensor.matmul(pa[64 * hh:64 * hh + 64, 0:IBLK],
                                          VT1[:, 64 * h:64 * h + 64],
                                          E1T[h][:], start=False, stop=True,
                                          tile_position=(0, 64 * hh))
                         rb = 32 * (h % 4)
                         rrow = RCB[h // 4][rb:rb + 1, :]
                         nc.tensor.matmul(prc[64 * hh:64 * hh + 64, 0:IBLK],
                                          ONESR[rb:rb + 1, 0:64], rrow,
                                          start=True, stop=True,
                                          tile_position=(rb, 64 * hh))
                     rcs = pDs.tile([128, IBLK], F32, tag="rcs")
                     nc.scalar.activation(rcs[:], prc[:, 0:IBLK], AF.Copy)
                     nc.vector.tensor_tensor(ANRM[hp][:], pa[:, 0:IBLK], rcs[:],
                                             op=ALU.mult)
             if DEBUG:
                 nc.sync.dma_start(dbgt('anrm0', (128, IBLK), BF16)[:],
                                   ANRM[0][:])

             with tc.tile_pool(name="psO", bufs=2, space="PSUM") as psO:
                 for mt in range(2):
                     po = psO.tile([128, 512], F32, tag="pso")
                     for kt in range(4):
                         nc.tensor.matmul(
                             po[:, 0:IBLK], WOT[kt][:, 128 * mt:128 * (mt + 1)],
                             ANRM[kt][:], start=(kt == 0), stop=(kt == 3))
                     ot = pD.tile([128, IBLK], F32, tag=f"ot{mt}",
                                  name=f"ot{mt}")
                     nc.scalar.activation(ot[:], po[:, 0:IBLK], AF.Identity,
                                          bias=BOUTR[:, mt:mt + 1])
                     nc.sync.dma_start(out_d[128 * mt:128 * (mt + 1), :],
                                       ot[:])

    nc.finalize()
    return nc, dbg


_build_cache = {}


def kernel(**inputs):
    shared = host_prep(inputs)
    cores = [host_core(inputs, c) for c in range(NCORE)]
    shapes = dict(shared)
    shapes.update(cores[0])

    key = 'nc'
    if key not in _build_cache:
        _build_cache[key] = build_nc(shapes)
    nc, dbg = _build_cache[key]

    in_maps = []
    for c in range(NCORE):
        m = dict(shared)
        m.update(cores[c])
        in_maps.append(m)

    trace = os.environ.get("KERNEL_TRACE", "0") == "1"
    res = run_bass_kernel_spmd(nc, in_maps, core_ids=list(range(NCORE)),
                               trace=trace)
    kernel.last_results = res
    out = np.zeros((1, DIM, 2501), np.float32)
    x1 = _f(inputs['x1'])
    out[0, :, 0] = x1[0, :, 0]          # cls token passes through
    full = np.concatenate([res.results[c]["out"] for c in range(NCORE)],
                          axis=1)       # (256, 2800)
    out[0, :, 1:] = full[:, :2500]
    return out.astype(np.float32)

